# revision 12
# baseline (speedup 1.0000x reference)
"""TRN2 Bass kernel for nn_LongTermAttention_18640158064770.

Sharding: 8 cores = (batch b in 0..3) x (query half qh in 0..1).

Key algebra (vs the reference):
1. scores [B,H,Q,NB] only feed scores@w_mu / scores@w_sigma -> tiny per-(b,h,q)
   Gaussian parameters (mu_q, sig2): computed on host in fp64 (~0.1% of FLOPs).
2. Gaussian semigroup smoothing: r[q,j] = N(mu_q; mu_j, sig2+sb_j^2) factors
   EXACTLY (up to ~e-13 quadrature error) as r = g @ S with
     S[j',j] = dlt * N(y_j'; mu_j, v0)      (fixed, folded into Gs on host)
     g[q,j'] = N(mu_q; y_j', sig2+sb^2-v0)  (per sigma-group s)
   on a coarse grid y of NJ=64 nodes spanning [-0.3, 1.3]. Then
     ctx = sum_s g_s @ (S_s @ vals) = g_cat @ vals_cat,   K = 2*NJ = 128.
   The smoothing also kills the ~176x cancellation in r@vals, so EVERY device
   matmul runs in fp16 (1 PE cycle/row vs fp32's 4) with rel err ~1.6e-3.
   Gs_cat is scaled by 2^10 (undone on host) to clear fp16 subnormals.
3. g is generated on-device: arg = A_s(q) + B_s(q) y + C_s(q) y^2 via a K=14
   matmul with hi/lo split fp16 coefficient rows (exact to ~2^-22), then Exp.

Per core the device computes (all matmuls fp16, PSUM fp32):
  A: Bm_cat[j',e] = sum_l Gs_cat[l,j'] k[l,e]        16 lt x 2 blocks
  T: Bm^T tiles via PE transpose (8x [128,128])
  B: vals_cat[j',e'] = sum_e Bm^T[e,j'] Wv^T[e,e']   8 et x 2 blocks
  R: arg[j'cat,q] = mks14^T @ stg14, Exp -> g        1 matmul + 1 ACT per (qb,h)
  C: ctx[d,q] = vals_cat[:,h*D:+D]^T @ g             1 matmul per (qb,h)
  O: out[q,e''] = ctxt^T @ Wo^T                      8-step accum per (qt,blk)
"""
import os
import numpy as np

import concourse.mybir as mybir
import concourse.tile as tile
from concourse import bacc
from concourse.bass_utils import run_bass_kernel_spmd


def _install_ntff_shim():
    """Provide antenv.axon_hooks so trace=True can capture NTFF profiles."""
    try:
        import sys, types
        import antenv
        if hasattr(antenv, "axon_hooks"):
            return
        from trn_agent_boot.trn_boot import _ntff_profile_via_ctypes
        mod = types.ModuleType("antenv.axon_hooks")
        _h = {"hook": _ntff_profile_via_ctypes("/opt/axon/libaxon_pjrt.so")}
        mod.get_axon_ntff_profile_hook = lambda: _h["hook"]
        mod.set_axon_ntff_profile_hook = lambda h: _h.__setitem__("hook", h)
        sys.modules["antenv.axon_hooks"] = mod
        antenv.axon_hooks = mod
    except Exception:
        pass


LAST_EXEC_NS = None

B, L, Q, H, D, NB = 4, 2048, 2048, 16, 64, 512
E = H * D
QC = Q // 2                 # queries per core
P = 128
SIGMAS = np.array([0.005, 0.01])
CLAMP_MIN = 1e-4

NJ, YLO, YHI = 64, -0.3, 1.3
DLT = (YHI - YLO) / (NJ - 1)
V0 = (2.2 * DLT) ** 2
SCALE = 1024.0
JC = 2 * NJ                 # 128 = stacked sigma groups
K14 = 14                    # split-coefficient contraction for g

F16 = mybir.dt.float16
F32 = mybir.dt.float32

_NC_CACHE = {}


def _build_nc():
    if "nc" in _NC_CACHE:
        return _NC_CACHE["nc"]
    nc = bacc.Bacc("TRN2", target_bir_lowering=False, debug=False)
    kb = nc.dram_tensor("kb", [L, E], F16, kind="ExternalInput")
    gsc = nc.dram_tensor("gsc", [L, JC], F16, kind="ExternalInput")
    wvt = nc.dram_tensor("wvt", [E, E], F16, kind="ExternalInput")
    wot = nc.dram_tensor("wot", [E, E], F16, kind="ExternalInput")
    stg = nc.dram_tensor("stg", [K14, 2 * H * 512], F16, kind="ExternalInput")
    mkp = nc.dram_tensor("mkp", [K14, JC], F16, kind="ExternalInput")
    idm = nc.dram_tensor("idm", [P, P], F16, kind="ExternalInput")
    out = nc.dram_tensor("out", [QC, E], F16, kind="ExternalOutput")

    ET, LT, QB = E // P, L // P, QC // 512

    with tile.TileContext(nc) as tc:
        with (
            tc.tile_pool(name="hold", bufs=1) as hold,
            tc.tile_pool(name="kst", bufs=LT) as kst,
            tc.tile_pool(name="gst", bufs=LT) as gst,
            tc.tile_pool(name="gp", bufs=34) as gp,
            tc.tile_pool(name="cxp", bufs=2) as cxp,
            tc.tile_pool(name="oev", bufs=3) as oev,
            tc.tile_pool(name="psa", bufs=2, space="PSUM") as psa,
            tc.tile_pool(name="psrc", bufs=4, space="PSUM") as psrc,
            tc.tile_pool(name="pstbo", bufs=2, space="PSUM") as pstbo,
        ):
            # ---- persistent SBUF ----
            mks = hold.tile([K14, JC], F16, tag="mks")
            stgs = hold.tile([K14, 2 * H * 512], F16, tag="stgs")
            wvs = hold.tile([P, ET * E], F16, tag="wvs")     # Wv.T: 8 x [128,1024]
            wos = hold.tile([P, ET * E], F16, tag="wos")     # Wo.T: 8 x [128,1024]
            bmj = hold.tile([P, E], F16, tag="bmj")          # Bm_cat [j', e]
            bme = hold.tile([P, ET * P], F16, tag="bme")     # Bm^T tiles [e, j']
            vals = hold.tile([P, E], F16, tag="vals")        # vals_cat [j', e']
            ident = hold.tile([P, P], F16, tag="ident")

            # ---- DMAs in priority order: R operands, k/gsc stream, weights.
            # Per-queue DMA bandwidth is ~1/16 of the core's total, so chunk
            # the critical tensors finely to spread them across queues.
            nc.sync.dma_start(out=mks[:], in_=mkp[:])
            SC = 2 * H * 512 // 16
            for c in range(16):
                nc.sync.dma_start(out=stgs[:, c * SC:(c + 1) * SC],
                                  in_=stg[:, c * SC:(c + 1) * SC])
            kts, gts_in = [], []
            for lt in range(LT):
                kt = kst.tile([P, E], F16, tag="kt")
                gt = gst.tile([P, JC], F16, tag="gt")
                nc.sync.dma_start(out=gt[:], in_=gsc[lt * P:(lt + 1) * P, :])
                for c in range(2):
                    nc.sync.dma_start(
                        out=kt[:, c * 512:(c + 1) * 512],
                        in_=kb[lt * P:(lt + 1) * P, c * 512:(c + 1) * 512])
                kts.append(kt); gts_in.append(gt)
            nc.sync.dma_start(out=ident[:], in_=idm[:])
            for et in range(ET):
                for c in range(2):
                    nc.sync.dma_start(
                        out=wvs[:, et * E + c * 512:et * E + (c + 1) * 512],
                        in_=wvt[et * P:(et + 1) * P, c * 512:(c + 1) * 512])
            for et in range(ET):
                for c in range(2):
                    nc.sync.dma_start(
                        out=wos[:, et * E + c * 512:et * E + (c + 1) * 512],
                        in_=wot[et * P:(et + 1) * P, c * 512:(c + 1) * 512])

            # ---- phase A (k-stream paced) interleaved with phase R ----
            pas = [psa.tile([P, 512], F32, tag="pa", name=f"pa{i}")
                   for i in range(2)]
            gts = []   # g tiles for all (qb, h), kept in SBUF

            def emit_r(i):
                pr = psrc.tile([P, 512], F32, tag="p")
                nc.tensor.matmul(pr[:], mks[:],
                                 stgs[:, i * 512:(i + 1) * 512],
                                 start=True, stop=True)
                g = gp.tile([P, 512], F16, tag="g")
                nc.scalar.activation(g[:], pr[:],
                                     mybir.ActivationFunctionType.Exp)
                gts.append(g)

            for lt in range(LT):
                emit_r(2 * lt)
                emit_r(2 * lt + 1)
                for blk in range(2):
                    nc.tensor.matmul(pas[blk][:], gts_in[lt][:],
                                     kts[lt][:, blk * 512:(blk + 1) * 512],
                                     start=(lt == 0), stop=(lt == LT - 1))
            for blk in range(2):
                nc.scalar.copy(bmj[:, blk * 512:(blk + 1) * 512], pas[blk][:])

            # ---- phase T: transpose Bm_cat to [e, j'] tiles (PE) ----
            for et in range(ET):
                pt = pstbo.tile([P, P], F16, tag="p")
                nc.tensor.transpose(pt[:], bmj[:, et * P:(et + 1) * P], ident[:])
                nc.vector.tensor_copy(bme[:, et * P:(et + 1) * P], pt[:])

            # ---- phase B: vals_cat[j', e'] ----
            for blk in range(2):
                pb = pstbo.tile([P, 512], F32, tag="p")
                for et in range(ET):
                    nc.tensor.matmul(
                        pb[:], bme[:, et * P:(et + 1) * P],
                        wvs[:, et * E + blk * 512: et * E + (blk + 1) * 512],
                        start=(et == 0), stop=(et == ET - 1))
                nc.scalar.copy(vals[:, blk * 512:(blk + 1) * 512], pb[:])

            # ---- phase C + O per q-block ----
            for qb in range(QB):
                ctxt = cxp.tile([P, ET * 512], F16, tag="ctxt")
                for h in range(H):
                    pc_ = psrc.tile([64, 512], F32, tag="p")
                    nc.tensor.matmul(pc_[:], vals[:, h * D:(h + 1) * D],
                                     gts[qb * H + h][:], start=True, stop=True)
                    et, off = h // 2, (h % 2) * D
                    dst = ctxt[off:off + D, et * 512:(et + 1) * 512]
                    if h % 2 == 0:
                        nc.vector.tensor_copy(dst, pc_[:])
                    else:
                        nc.scalar.copy(dst, pc_[:])
                for qt in range(4):
                    for blk in range(2):
                        po = pstbo.tile([P, 512], F32, tag="p")
                        for et in range(ET):
                            nc.tensor.matmul(
                                po[:],
                                ctxt[:, et * 512 + qt * P: et * 512 + (qt + 1) * P],
                                wos[:, et * E + blk * 512: et * E + (blk + 1) * 512],
                                start=(et == 0), stop=(et == ET - 1))
                        ot = oev.tile([P, 512], F16, tag="ot")
                        nc.vector.tensor_copy(ot[:], po[:])
                        nc.sync.dma_start(
                            out=out[qb * 512 + qt * P: qb * 512 + (qt + 1) * P,
                                    blk * 512:(blk + 1) * 512],
                            in_=ot[:])
    nc.compile()
    _NC_CACHE["nc"] = nc
    return nc


def _f16(x):
    return np.ascontiguousarray(np.asarray(x, np.float16))


def _host_prep(k, q, Wq, Wk, w_mu, w_sigma, Gs, basis_mu):
    """fp64 host prep: Gaussian params per (b,q,h), S-fold, split coef rows."""
    f8 = np.float64
    sD = 1.0 / np.sqrt(f8(D))
    k8, q8, Gs8 = k.astype(f8), q.astype(f8), Gs.astype(f8)
    mu8 = basis_mu.astype(f8)

    # fp16-exact grid nodes + basis rows
    y = np.linspace(YLO, YHI, NJ).astype(np.float16).astype(f8)
    u2 = y * y
    u2h = np.float16(u2).astype(f8)
    u2l = np.float16(u2 - u2h).astype(f8)
    ones = np.ones(NJ)
    mks = np.zeros((K14, JC), np.float16)
    blk = np.stack([ones, ones, y, y, u2h, u2h, u2l])
    for s in range(2):
        mks[s * 7:(s + 1) * 7, s * NJ:(s + 1) * NJ] = blk.astype(np.float16)

    # S fold (per sigma group; original basis order has sigma tiled/alternating)
    Scat = np.zeros((JC, NB))
    for s in range(2):
        js = np.arange(s, NB, 2)
        Scat[s * NJ:(s + 1) * NJ, js] = (
            DLT * np.exp(-0.5 * (y[:, None] - mu8[None, js]) ** 2 / V0)
            / np.sqrt(2 * np.pi * V0))
    gs_cat = _f16((Gs8 @ Scat.T) * SCALE)            # [L, JC]

    # scalar path: mu_q, sig2 per (b, q, h)
    g2 = Gs8 @ np.stack([w_mu.astype(f8), w_sigma.astype(f8)], 1)   # [L,2]
    stg_all = np.empty((B, 2, K14, 2 * H * 512), np.float16)
    for b in range(B):
        t = k8[b].T @ g2                                            # [E,2]
        Wh = np.empty((E, H, 2), f8)
        for h in range(H):
            u_ = Wk.astype(f8)[h * D:(h + 1) * D, :] @ t * sD
            Wh[:, h, :] = Wq.astype(f8)[h * D:(h + 1) * D, :].T @ u_
        sv = np.einsum('qe,ehc->qhc', q8[b], Wh)                    # [Q,H,2]
        mu = 1.0 / (1.0 + np.exp(-sv[..., 0]))                      # [Q,H]
        sig2 = np.clip(np.logaddexp(0.0, sv[..., 1]), CLAMP_MIN, None)
        rows = np.empty((K14, Q, H), f8)
        for s in range(2):
            var = sig2 + SIGMAS[s] ** 2 - V0
            Ac = -0.5 * mu * mu / var - 0.5 * np.log(2 * np.pi * var)
            Bc = mu / var
            Cc = -0.5 / var
            Ah = np.float16(Ac).astype(f8); Al = Ac - Ah
            Bh = np.float16(Bc).astype(f8); Bl = Bc - Bh
            Ch = np.float16(Cc).astype(f8); Cl = Cc - Ch
            rows[s * 7:(s + 1) * 7] = np.stack([Ah, Al, Bh, Bl, Ch, Cl, Ch])
        r16 = rows.astype(np.float16)                               # [14, Q, H]
        # stg col = (qb*H + h)*512 + i ; q index within core = qb*512 + i
        for qh in range(2):
            for qb in range(QC // 512):
                sl = r16[:, qh * QC + qb * 512: qh * QC + (qb + 1) * 512, :]
                stg_all[b, qh, :, qb * H * 512:(qb + 1) * H * 512] = (
                    sl.transpose(0, 2, 1).reshape(K14, H * 512))
    return gs_cat, mks, stg_all


def kernel(k, q, Wq, Wk, Wv, Wo, w_mu, w_sigma, Gs, basis_mu, basis_sigma):
    k = np.ascontiguousarray(np.asarray(k, np.float32))
    q = np.ascontiguousarray(np.asarray(q, np.float32))
    gs_cat, mks, stg_all = _host_prep(
        k, q, np.asarray(Wq), np.asarray(Wk),
        np.asarray(w_mu), np.asarray(w_sigma),
        np.asarray(Gs), np.asarray(basis_mu))
    wvt = _f16(np.asarray(Wv, np.float32).T)
    wot = _f16(np.asarray(Wo, np.float32).T)

    nc = _build_nc()
    in_maps = []
    for c in range(8):
        b, qh = c // 2, c % 2
        in_maps.append({
            "kb": _f16(k[b]), "gsc": gs_cat, "wvt": wvt, "wot": wot,
            "stg": np.ascontiguousarray(stg_all[b, qh]),
            "mkp": mks, "idm": np.eye(P, dtype=np.float16),
        })
    trace = bool(os.environ.get("KERNEL_TRACE"))
    if trace:
        _install_ntff_shim()
    res = run_bass_kernel_spmd(nc, in_maps, list(range(8)), trace=trace)
    global LAST_EXEC_NS
    LAST_EXEC_NS = res.exec_time_ns
    out = np.empty((B, Q, E), np.float32)
    for c in range(8):
        b, qh = c // 2, c % 2
        out[b, qh * QC:(qh + 1) * QC, :] = res.results[c]["out"].astype(np.float32)
    out *= np.float32(1.0 / SCALE)
    return out


# revision 15
# speedup vs baseline: 1.1963x; 1.1963x over previous
"""TRN2 Bass kernel for nn_LongTermAttention_18640158064770.

Sharding: 8 cores = (batch b in 0..3) x (query half qh in 0..1).

Key algebra (vs the reference):
1. scores [B,H,Q,NB] only feed scores@w_mu / scores@w_sigma -> tiny per-(b,h,q)
   Gaussian parameters (mu_q, sig2): computed on host in fp64 (~0.1% of FLOPs).
2. Gaussian semigroup smoothing: r[q,j] = N(mu_q; mu_j, sig2+sb_j^2) factors
   EXACTLY (up to ~e-13 quadrature error) as r = g @ S with
     S[j',j] = dlt * N(y_j'; mu_j, v0)      (fixed, folded into Gs on host)
     g[q,j'] = N(mu_q; y_j', sig2+sb^2-v0)  (per sigma-group s)
   on a coarse grid y of NJ=64 nodes spanning [-0.3, 1.3]. Then
     ctx = sum_s g_s @ (S_s @ vals) = g_cat @ vals_cat,   K = 2*NJ = 128.
   The smoothing also kills the ~176x cancellation in r@vals, so EVERY device
   matmul runs in fp16 (1 PE cycle/row vs fp32's 4) with rel err ~1.6e-3.
   Gs_cat is scaled by 2^10 (undone on host) to clear fp16 subnormals.
3. g is generated on-device: arg = A_s(q) + B_s(q) y + C_s(q) y^2 via a K=14
   matmul with hi/lo split fp16 coefficient rows (exact to ~2^-22), then Exp.

Per core the device computes (all matmuls fp16, PSUM fp32):
  A: Bm_cat[j',e] = sum_l Gs_cat[l,j'] k[l,e]        16 lt x 2 blocks
  T: Bm^T tiles via PE transpose (8x [128,128])
  B: vals_cat[j',e'] = sum_e Bm^T[e,j'] Wv^T[e,e']   8 et x 2 blocks
  R: arg[j'cat,q] = mks14^T @ stg14, Exp -> g        1 matmul + 1 ACT per (qb,h)
  C: ctx[d,q] = vals_cat[:,h*D:+D]^T @ g             1 matmul per (qb,h)
  O: out[q,e''] = ctxt^T @ Wo^T                      8-step accum per (qt,blk)
"""
import os
import numpy as np

import concourse.mybir as mybir
import concourse.tile as tile
from concourse import bacc
from concourse.bass_utils import run_bass_kernel_spmd


def _install_ntff_shim():
    """Provide antenv.axon_hooks so trace=True can capture NTFF profiles."""
    try:
        import sys, types
        import antenv
        if hasattr(antenv, "axon_hooks"):
            return
        from trn_agent_boot.trn_boot import _ntff_profile_via_ctypes
        mod = types.ModuleType("antenv.axon_hooks")
        _h = {"hook": _ntff_profile_via_ctypes("/opt/axon/libaxon_pjrt.so")}
        mod.get_axon_ntff_profile_hook = lambda: _h["hook"]
        mod.set_axon_ntff_profile_hook = lambda h: _h.__setitem__("hook", h)
        sys.modules["antenv.axon_hooks"] = mod
        antenv.axon_hooks = mod
    except Exception:
        pass


LAST_EXEC_NS = None

B, L, Q, H, D, NB = 4, 2048, 2048, 16, 64, 512
E = H * D
QC = Q // 2                 # queries per core
P = 128
SIGMAS = np.array([0.005, 0.01])
CLAMP_MIN = 1e-4

NJ, YLO, YHI = 64, -0.3, 1.3
DLT = (YHI - YLO) / (NJ - 1)
V0 = (2.2 * DLT) ** 2
SCALE = 1024.0
JC = 2 * NJ                 # 128 = stacked sigma groups
K14 = 14                    # split-coefficient contraction for g

F16 = mybir.dt.float16
F32 = mybir.dt.float32

_NC_CACHE = {}


def _build_nc():
    if "nc" in _NC_CACHE:
        return _NC_CACHE["nc"]
    nc = bacc.Bacc("TRN2", target_bir_lowering=False, debug=False)
    kb = nc.dram_tensor("kb", [L, E], F16, kind="ExternalInput")
    gsc = nc.dram_tensor("gsc", [L, JC], F16, kind="ExternalInput")
    wvt = nc.dram_tensor("wvt", [E, E], F16, kind="ExternalInput")
    wot = nc.dram_tensor("wot", [E, E], F16, kind="ExternalInput")
    stg = nc.dram_tensor("stg", [K14, 2 * H * 512], F16, kind="ExternalInput")
    mkp = nc.dram_tensor("mkp", [K14, JC], F16, kind="ExternalInput")
    idm = nc.dram_tensor("idm", [P, P], F16, kind="ExternalInput")
    out = nc.dram_tensor("out", [QC, E], F16, kind="ExternalOutput")

    ET, LT, QB = E // P, L // P, QC // 512

    with tile.TileContext(nc) as tc:
        with (
            tc.tile_pool(name="hold", bufs=1) as hold,
            tc.tile_pool(name="gp", bufs=34) as gp,
            tc.tile_pool(name="cxp", bufs=2) as cxp,
            tc.tile_pool(name="oev", bufs=2) as oev,
            tc.tile_pool(name="psa", bufs=2, space="PSUM") as psa,
            tc.tile_pool(name="psrc", bufs=4, space="PSUM") as psrc,
            tc.tile_pool(name="pstbo", bufs=2, space="PSUM") as pstbo,
        ):
            # ---- persistent SBUF ----
            mks = hold.tile([K14, JC], F16, tag="mks")
            stgs = hold.tile([K14, 2 * H * 512], F16, tag="stgs")
            kall = hold.tile([P, LT * E], F16, tag="kall")   # k: 16 x [128,1024]
            gall = hold.tile([P, LT * JC], F16, tag="gall")  # Gs_cat l-tiles
            wvs = hold.tile([P, ET * E], F16, tag="wvs")     # Wv.T: 8 x [128,1024]
            wos = hold.tile([P, ET * E], F16, tag="wos")     # Wo.T: 8 x [128,1024]
            bmj = hold.tile([P, E], F16, tag="bmj")          # Bm_cat [j', e]
            bme = hold.tile([P, ET * P], F16, tag="bme")     # Bm^T tiles [e, j']
            vals = hold.tile([P, E], F16, tag="vals")        # vals_cat [j', e']
            ident = hold.tile([P, P], F16, tag="ident")

            # ---- batched DMAs, demand-ordered. Each dma_start costs ~800ns
            # of serialized dispatch on the sync sequencer, so use FEW large
            # transfers (descriptors stripe across all 16 queues).
            nc.sync.dma_start(out=mks[:], in_=mkp[:])
            nc.sync.dma_start(out=ident[:], in_=idm[:])
            nc.sync.dma_start(out=stgs[:], in_=stg[:])

            def load_tiled(dst, dsrc, n_tiles, width):
                # dsrc [(n_tiles p), width] -> dst [p, (n_tiles width)]
                nc.sync.dma_start(
                    out=dst.rearrange("p (t w) -> p t w", t=n_tiles),
                    in_=dsrc.rearrange("(t p) w -> p t w", p=P))

            # k chunk 0 (4 l-tiles), gsc, Wv, then rest of k, then Wo
            nc.sync.dma_start(
                out=kall[:, 0:4 * E].rearrange("p (t w) -> p t w", t=4),
                in_=kb[0:512, :].rearrange("(t p) w -> p t w", p=P))
            load_tiled(gall[:, :], gsc[:, :], LT, JC)
            load_tiled(wvs[:, :], wvt[:, :], ET, E)
            for c in range(1, 4):
                nc.sync.dma_start(
                    out=kall[:, c * 4 * E:(c + 1) * 4 * E].rearrange(
                        "p (t w) -> p t w", t=4),
                    in_=kb[c * 512:(c + 1) * 512, :].rearrange(
                        "(t p) w -> p t w", p=P))
            load_tiled(wos[:, :], wot[:, :], ET, E)

            # ---- phase A (k-stream paced) interleaved with phase R ----
            pas = [psa.tile([P, 512], F32, tag="pa", name=f"pa{i}")
                   for i in range(2)]
            gts = []   # g tiles for all (qb, h), kept in SBUF

            def emit_r(i):
                pr = psrc.tile([P, 512], F32, tag="p")
                nc.tensor.matmul(pr[:], mks[:],
                                 stgs[:, i * 512:(i + 1) * 512],
                                 start=True, stop=True)
                g = gp.tile([P, 512], F16, tag="g")
                nc.scalar.activation(g[:], pr[:],
                                     mybir.ActivationFunctionType.Exp)
                gts.append(g)

            for lt in range(LT):
                emit_r(2 * lt)
                emit_r(2 * lt + 1)
                for blk in range(2):
                    nc.tensor.matmul(
                        pas[blk][:], gall[:, lt * JC:(lt + 1) * JC],
                        kall[:, lt * E + blk * 512: lt * E + (blk + 1) * 512],
                        start=(lt == 0), stop=(lt == LT - 1))
            for blk in range(2):
                nc.vector.tensor_copy(bmj[:, blk * 512:(blk + 1) * 512],
                                      pas[blk][:])

            # ---- phase T: transpose Bm_cat to [e, j'] tiles (PE) ----
            for et in range(ET):
                pt = pstbo.tile([P, P], F16, tag="p")
                nc.tensor.transpose(pt[:], bmj[:, et * P:(et + 1) * P], ident[:])
                nc.vector.tensor_copy(bme[:, et * P:(et + 1) * P], pt[:])

            # ---- phase B: vals_cat[j', e'] ----
            for blk in range(2):
                pb = pstbo.tile([P, 512], F32, tag="p")
                for et in range(ET):
                    nc.tensor.matmul(
                        pb[:], bme[:, et * P:(et + 1) * P],
                        wvs[:, et * E + blk * 512: et * E + (blk + 1) * 512],
                        start=(et == 0), stop=(et == ET - 1))
                nc.vector.tensor_copy(vals[:, blk * 512:(blk + 1) * 512], pb[:])

            # ---- phase C + O per q-block ----
            for qb in range(QB):
                ctxt = cxp.tile([P, ET * 512], F16, tag="ctxt")
                for h in range(H):
                    pc_ = psrc.tile([64, 512], F32, tag="p")
                    nc.tensor.matmul(pc_[:], vals[:, h * D:(h + 1) * D],
                                     gts[qb * H + h][:], start=True, stop=True)
                    et, off = h // 2, (h % 2) * D
                    dst = ctxt[off:off + D, et * 512:(et + 1) * 512]
                    if h % 2 == 0:
                        nc.vector.tensor_copy(dst, pc_[:])
                    else:
                        nc.scalar.copy(dst, pc_[:])
                oall = oev.tile([P, 8 * 512], F16, tag="oall")
                for qt in range(4):
                    for blk in range(2):
                        po = pstbo.tile([P, 512], F32, tag="p")
                        for et in range(ET):
                            nc.tensor.matmul(
                                po[:],
                                ctxt[:, et * 512 + qt * P: et * 512 + (qt + 1) * P],
                                wos[:, et * E + blk * 512: et * E + (blk + 1) * 512],
                                start=(et == 0), stop=(et == ET - 1))
                        dst = oall[:, (qt * 2 + blk) * 512:(qt * 2 + blk + 1) * 512]
                        if blk == 0:
                            nc.vector.tensor_copy(dst, po[:])
                        else:
                            nc.scalar.copy(dst, po[:])
                nc.sync.dma_start(
                    out=out[qb * 512:(qb + 1) * 512, :].rearrange(
                        "(t p) (b c) -> p t b c", p=P, b=2),
                    in_=oall[:, :].rearrange("p (t b c) -> p t b c", t=4, b=2))
    nc.compile()
    _NC_CACHE["nc"] = nc
    return nc


def _f16(x):
    return np.ascontiguousarray(np.asarray(x, np.float16))


def _host_prep(k, q, Wq, Wk, w_mu, w_sigma, Gs, basis_mu):
    """fp64 host prep: Gaussian params per (b,q,h), S-fold, split coef rows."""
    f8 = np.float64
    sD = 1.0 / np.sqrt(f8(D))
    k8, q8, Gs8 = k.astype(f8), q.astype(f8), Gs.astype(f8)
    mu8 = basis_mu.astype(f8)

    # fp16-exact grid nodes + basis rows
    y = np.linspace(YLO, YHI, NJ).astype(np.float16).astype(f8)
    u2 = y * y
    u2h = np.float16(u2).astype(f8)
    u2l = np.float16(u2 - u2h).astype(f8)
    ones = np.ones(NJ)
    mks = np.zeros((K14, JC), np.float16)
    blk = np.stack([ones, ones, y, y, u2h, u2h, u2l])
    for s in range(2):
        mks[s * 7:(s + 1) * 7, s * NJ:(s + 1) * NJ] = blk.astype(np.float16)

    # S fold (per sigma group; original basis order has sigma tiled/alternating)
    Scat = np.zeros((JC, NB))
    for s in range(2):
        js = np.arange(s, NB, 2)
        Scat[s * NJ:(s + 1) * NJ, js] = (
            DLT * np.exp(-0.5 * (y[:, None] - mu8[None, js]) ** 2 / V0)
            / np.sqrt(2 * np.pi * V0))
    gs_cat = _f16((Gs8 @ Scat.T) * SCALE)            # [L, JC]

    # scalar path: mu_q, sig2 per (b, q, h)
    g2 = Gs8 @ np.stack([w_mu.astype(f8), w_sigma.astype(f8)], 1)   # [L,2]
    stg_all = np.empty((B, 2, K14, 2 * H * 512), np.float16)
    for b in range(B):
        t = k8[b].T @ g2                                            # [E,2]
        Wh = np.empty((E, H, 2), f8)
        for h in range(H):
            u_ = Wk.astype(f8)[h * D:(h + 1) * D, :] @ t * sD
            Wh[:, h, :] = Wq.astype(f8)[h * D:(h + 1) * D, :].T @ u_
        sv = np.einsum('qe,ehc->qhc', q8[b], Wh)                    # [Q,H,2]
        mu = 1.0 / (1.0 + np.exp(-sv[..., 0]))                      # [Q,H]
        sig2 = np.clip(np.logaddexp(0.0, sv[..., 1]), CLAMP_MIN, None)
        rows = np.empty((K14, Q, H), f8)
        for s in range(2):
            var = sig2 + SIGMAS[s] ** 2 - V0
            Ac = -0.5 * mu * mu / var - 0.5 * np.log(2 * np.pi * var)
            Bc = mu / var
            Cc = -0.5 / var
            Ah = np.float16(Ac).astype(f8); Al = Ac - Ah
            Bh = np.float16(Bc).astype(f8); Bl = Bc - Bh
            Ch = np.float16(Cc).astype(f8); Cl = Cc - Ch
            rows[s * 7:(s + 1) * 7] = np.stack([Ah, Al, Bh, Bl, Ch, Cl, Ch])
        r16 = rows.astype(np.float16)                               # [14, Q, H]
        # stg col = (qb*H + h)*512 + i ; q index within core = qb*512 + i
        for qh in range(2):
            for qb in range(QC // 512):
                sl = r16[:, qh * QC + qb * 512: qh * QC + (qb + 1) * 512, :]
                stg_all[b, qh, :, qb * H * 512:(qb + 1) * H * 512] = (
                    sl.transpose(0, 2, 1).reshape(K14, H * 512))
    return gs_cat, mks, stg_all


def kernel(k, q, Wq, Wk, Wv, Wo, w_mu, w_sigma, Gs, basis_mu, basis_sigma):
    k = np.ascontiguousarray(np.asarray(k, np.float32))
    q = np.ascontiguousarray(np.asarray(q, np.float32))
    gs_cat, mks, stg_all = _host_prep(
        k, q, np.asarray(Wq), np.asarray(Wk),
        np.asarray(w_mu), np.asarray(w_sigma),
        np.asarray(Gs), np.asarray(basis_mu))
    wvt = _f16(np.asarray(Wv, np.float32).T)
    wot = _f16(np.asarray(Wo, np.float32).T)

    nc = _build_nc()
    in_maps = []
    for c in range(8):
        b, qh = c // 2, c % 2
        in_maps.append({
            "kb": _f16(k[b]), "gsc": gs_cat, "wvt": wvt, "wot": wot,
            "stg": np.ascontiguousarray(stg_all[b, qh]),
            "mkp": mks, "idm": np.eye(P, dtype=np.float16),
        })
    trace = bool(os.environ.get("KERNEL_TRACE"))
    if trace:
        _install_ntff_shim()
    res = run_bass_kernel_spmd(nc, in_maps, list(range(8)), trace=trace)
    global LAST_EXEC_NS
    LAST_EXEC_NS = res.exec_time_ns
    out = np.empty((B, Q, E), np.float32)
    for c in range(8):
        b, qh = c // 2, c % 2
        out[b, qh * QC:(qh + 1) * QC, :] = res.results[c]["out"].astype(np.float32)
    out *= np.float32(1.0 / SCALE)
    return out


# revision 17
# speedup vs baseline: 1.2849x; 1.0741x over previous
"""TRN2 Bass kernel for nn_LongTermAttention_18640158064770.

Sharding: 8 cores = (batch b in 0..3) x (query half qh in 0..1).

Key algebra (vs the reference):
1. scores [B,H,Q,NB] only feed scores@w_mu / scores@w_sigma -> tiny per-(b,h,q)
   Gaussian parameters (mu_q, sig2): computed on host in fp64 (~0.1% of FLOPs).
2. Gaussian semigroup smoothing: r[q,j] = N(mu_q; mu_j, sig2+sb_j^2) factors
   EXACTLY (up to ~e-13 quadrature error) as r = g @ S with
     S[j',j] = dlt * N(y_j'; mu_j, v0)      (fixed, folded into Gs on host)
     g[q,j'] = N(mu_q; y_j', sig2+sb^2-v0)  (per sigma-group s)
   on a coarse grid y of NJ=64 nodes spanning [-0.3, 1.3]. Then
     ctx = sum_s g_s @ (S_s @ vals) = g_cat @ vals_cat,   K = 2*NJ = 128.
   The smoothing also kills the ~176x cancellation in r@vals, so EVERY device
   matmul runs in fp16 (1 PE cycle/row vs fp32's 4) with rel err ~1.6e-3.
   Gs_cat is scaled by 2^10 (undone on host) to clear fp16 subnormals.
3. g is generated on-device: arg = A_s(q) + B_s(q) y + C_s(q) y^2 via a K=14
   matmul with hi/lo split fp16 coefficient rows (exact to ~2^-22), then Exp.

Per core the device computes (all matmuls fp16, PSUM fp32):
  A: Bm_cat[j',e] = sum_l Gs_cat[l,j'] k[l,e]        16 lt x 2 blocks
  T: Bm^T tiles via PE transpose (8x [128,128])
  B: vals_cat[j',e'] = sum_e Bm^T[e,j'] Wv^T[e,e']   8 et x 2 blocks
  R: arg[j'cat,q] = mks14^T @ stg14, Exp -> g        1 matmul + 1 ACT per (qb,h)
  C: ctx[d,q] = vals_cat[:,h*D:+D]^T @ g             1 matmul per (qb,h)
  O: out[q,e''] = ctxt^T @ Wo^T                      8-step accum per (qt,blk)
"""
import os
import numpy as np

import concourse.mybir as mybir
import concourse.tile as tile
from concourse import bacc
from concourse.bass_utils import run_bass_kernel_spmd


def _install_ntff_shim():
    """Provide antenv.axon_hooks so trace=True can capture NTFF profiles."""
    try:
        import sys, types
        import antenv
        if hasattr(antenv, "axon_hooks"):
            return
        from trn_agent_boot.trn_boot import _ntff_profile_via_ctypes
        mod = types.ModuleType("antenv.axon_hooks")
        _h = {"hook": _ntff_profile_via_ctypes("/opt/axon/libaxon_pjrt.so")}
        mod.get_axon_ntff_profile_hook = lambda: _h["hook"]
        mod.set_axon_ntff_profile_hook = lambda h: _h.__setitem__("hook", h)
        sys.modules["antenv.axon_hooks"] = mod
        antenv.axon_hooks = mod
    except Exception:
        pass


LAST_EXEC_NS = None

B, L, Q, H, D, NB = 4, 2048, 2048, 16, 64, 512
E = H * D
QC = Q // 2                 # queries per core
P = 128
SIGMAS = np.array([0.005, 0.01])
CLAMP_MIN = 1e-4

NJ, YLO, YHI = 64, -0.3, 1.3
DLT = (YHI - YLO) / (NJ - 1)
V0 = (2.2 * DLT) ** 2
SCALE = 1024.0
JC = 2 * NJ                 # 128 = stacked sigma groups
K14 = 14                    # split-coefficient contraction for g

F16 = mybir.dt.float16
F32 = mybir.dt.float32

_NC_CACHE = {}


def _build_nc():
    if "nc" in _NC_CACHE:
        return _NC_CACHE["nc"]
    nc = bacc.Bacc("TRN2", target_bir_lowering=False, debug=False)
    kb = nc.dram_tensor("kb", [L, E], F16, kind="ExternalInput")
    gsc = nc.dram_tensor("gsc", [L, JC], F16, kind="ExternalInput")
    wvt = nc.dram_tensor("wvt", [E, E], F16, kind="ExternalInput")
    wot = nc.dram_tensor("wot", [E, E], F16, kind="ExternalInput")
    stg = nc.dram_tensor("stg", [K14, 2 * H * 512], F16, kind="ExternalInput")
    mkp = nc.dram_tensor("mkp", [K14, JC], F16, kind="ExternalInput")
    idm = nc.dram_tensor("idm", [P, P], F16, kind="ExternalInput")
    out = nc.dram_tensor("out", [QC, E], F16, kind="ExternalOutput")

    ET, LT, QB = E // P, L // P, QC // 512

    with tile.TileContext(nc) as tc:
        with (
            tc.tile_pool(name="hold", bufs=1) as hold,
            tc.tile_pool(name="gp", bufs=34) as gp,
            tc.tile_pool(name="cxp", bufs=2) as cxp,
            tc.tile_pool(name="oev", bufs=2) as oev,
            tc.tile_pool(name="psa", bufs=2, space="PSUM") as psa,
            tc.tile_pool(name="psrc", bufs=4, space="PSUM") as psrc,
            tc.tile_pool(name="pstbo", bufs=2, space="PSUM") as pstbo,
        ):
            # ---- persistent SBUF ----
            mks = hold.tile([K14, JC], F16, tag="mks")
            stgs = hold.tile([K14, 2 * H * 512], F16, tag="stgs")
            kall = hold.tile([P, LT * E], F16, tag="kall")   # k: 16 x [128,1024]
            gall = hold.tile([P, LT * JC], F16, tag="gall")  # Gs_cat l-tiles
            wvs = hold.tile([P, ET * E], F16, tag="wvs")     # Wv.T: 8 x [128,1024]
            wos = hold.tile([P, ET * E], F16, tag="wos")     # Wo.T: 8 x [128,1024]
            bmj = hold.tile([P, E], F16, tag="bmj")          # Bm_cat [j', e]
            bme = hold.tile([P, ET * P], F16, tag="bme")     # Bm^T tiles [e, j']
            vals = hold.tile([P, E], F16, tag="vals")        # vals_cat [j', e']
            ident = hold.tile([P, P], F16, tag="ident")

            # ---- batched DMAs, demand-ordered. Each dma_start costs ~800ns
            # of serialized dispatch on the sync sequencer, so use FEW large
            # transfers (descriptors stripe across all 16 queues).
            nc.sync.dma_start(out=stgs[:], in_=stg[:])
            nc.sync.dma_start(out=mks[:], in_=mkp[:])
            nc.sync.dma_start(out=ident[:], in_=idm[:])

            def load_tiled(dst, dsrc, n_tiles, width):
                # dsrc [(n_tiles p), width] -> dst [p, (n_tiles width)]
                nc.sync.dma_start(
                    out=dst.rearrange("p (t w) -> p t w", t=n_tiles),
                    in_=dsrc.rearrange("(t p) w -> p t w", p=P))

            # k + gsc (phase A stream) first, then Wv, then Wo
            nc.sync.dma_start(
                out=kall[:, 0:4 * E].rearrange("p (t w) -> p t w", t=4),
                in_=kb[0:512, :].rearrange("(t p) w -> p t w", p=P))
            load_tiled(gall[:, :], gsc[:, :], LT, JC)
            for c in range(1, 4):
                nc.sync.dma_start(
                    out=kall[:, c * 4 * E:(c + 1) * 4 * E].rearrange(
                        "p (t w) -> p t w", t=4),
                    in_=kb[c * 512:(c + 1) * 512, :].rearrange(
                        "(t p) w -> p t w", p=P))
            load_tiled(wvs[:, :], wvt[:, :], ET, E)
            load_tiled(wos[:, :], wot[:, :], ET, E)

            # ---- phase A (k-stream paced) interleaved with phase R ----
            pas = [psa.tile([P, 512], F32, tag="pa", name=f"pa{i}")
                   for i in range(2)]
            gts = []   # g tiles for all (qb, h), kept in SBUF

            def emit_r(i):
                pr = psrc.tile([P, 512], F32, tag="p")
                nc.tensor.matmul(pr[:], mks[:],
                                 stgs[:, i * 512:(i + 1) * 512],
                                 start=True, stop=True)
                g = gp.tile([P, 512], F16, tag="g")
                nc.scalar.activation(g[:], pr[:],
                                     mybir.ActivationFunctionType.Exp)
                gts.append(g)

            for lt in range(LT):
                emit_r(2 * lt)
                emit_r(2 * lt + 1)
                for blk in range(2):
                    nc.tensor.matmul(
                        pas[blk][:], gall[:, lt * JC:(lt + 1) * JC],
                        kall[:, lt * E + blk * 512: lt * E + (blk + 1) * 512],
                        start=(lt == 0), stop=(lt == LT - 1))
            for blk in range(2):
                nc.vector.tensor_copy(bmj[:, blk * 512:(blk + 1) * 512],
                                      pas[blk][:])

            # ---- phase T: transpose Bm_cat to [e, j'] tiles (PE) ----
            for et in range(ET):
                pt = pstbo.tile([P, P], F16, tag="p")
                nc.tensor.transpose(pt[:], bmj[:, et * P:(et + 1) * P], ident[:])
                nc.vector.tensor_copy(bme[:, et * P:(et + 1) * P], pt[:])

            # ---- phase B: vals_cat[j', e'] ----
            for blk in range(2):
                pb = pstbo.tile([P, 512], F32, tag="p")
                for et in range(ET):
                    nc.tensor.matmul(
                        pb[:], bme[:, et * P:(et + 1) * P],
                        wvs[:, et * E + blk * 512: et * E + (blk + 1) * 512],
                        start=(et == 0), stop=(et == ET - 1))
                nc.vector.tensor_copy(vals[:, blk * 512:(blk + 1) * 512], pb[:])

            # ---- phase C + O per q-block ----
            for qb in range(QB):
                ctxt = cxp.tile([P, ET * 512], F16, tag="ctxt")
                for h in range(H):
                    pc_ = psrc.tile([64, 512], F32, tag="p")
                    nc.tensor.matmul(pc_[:], vals[:, h * D:(h + 1) * D],
                                     gts[qb * H + h][:], start=True, stop=True)
                    et, off = h // 2, (h % 2) * D
                    dst = ctxt[off:off + D, et * 512:(et + 1) * 512]
                    if h % 2 == 0:
                        nc.vector.tensor_copy(dst, pc_[:])
                    else:
                        nc.scalar.copy(dst, pc_[:])
                oall = oev.tile([P, 8 * 512], F16, tag="oall")
                for qt in range(4):
                    for blk in range(2):
                        po = pstbo.tile([P, 512], F32, tag="p")
                        for et in range(ET):
                            nc.tensor.matmul(
                                po[:],
                                ctxt[:, et * 512 + qt * P: et * 512 + (qt + 1) * P],
                                wos[:, et * E + blk * 512: et * E + (blk + 1) * 512],
                                start=(et == 0), stop=(et == ET - 1))
                        dst = oall[:, (qt * 2 + blk) * 512:(qt * 2 + blk + 1) * 512]
                        if blk == 0:
                            nc.vector.tensor_copy(dst, po[:])
                        else:
                            nc.scalar.copy(dst, po[:])
                    if qt % 2 == 1:
                        # store per 2 q-tiles to shrink the end-of-kernel tail
                        q0 = qb * 512 + (qt - 1) * P
                        nc.sync.dma_start(
                            out=out[q0:q0 + 2 * P, :].rearrange(
                                "(t p) (b c) -> p t b c", p=P, b=2),
                            in_=oall[:, (qt - 1) * 1024:(qt + 1) * 1024].rearrange(
                                "p (t b c) -> p t b c", t=2, b=2))
    nc.compile()
    _NC_CACHE["nc"] = nc
    return nc


def _f16(x):
    return np.ascontiguousarray(np.asarray(x, np.float16))


def _host_prep(k, q, Wq, Wk, w_mu, w_sigma, Gs, basis_mu):
    """fp64 host prep: Gaussian params per (b,q,h), S-fold, split coef rows."""
    f8 = np.float64
    sD = 1.0 / np.sqrt(f8(D))
    k8, q8, Gs8 = k.astype(f8), q.astype(f8), Gs.astype(f8)
    mu8 = basis_mu.astype(f8)

    # fp16-exact grid nodes + basis rows
    y = np.linspace(YLO, YHI, NJ).astype(np.float16).astype(f8)
    u2 = y * y
    u2h = np.float16(u2).astype(f8)
    u2l = np.float16(u2 - u2h).astype(f8)
    ones = np.ones(NJ)
    mks = np.zeros((K14, JC), np.float16)
    blk = np.stack([ones, ones, y, y, u2h, u2h, u2l])
    for s in range(2):
        mks[s * 7:(s + 1) * 7, s * NJ:(s + 1) * NJ] = blk.astype(np.float16)

    # S fold (per sigma group; original basis order has sigma tiled/alternating)
    Scat = np.zeros((JC, NB))
    for s in range(2):
        js = np.arange(s, NB, 2)
        Scat[s * NJ:(s + 1) * NJ, js] = (
            DLT * np.exp(-0.5 * (y[:, None] - mu8[None, js]) ** 2 / V0)
            / np.sqrt(2 * np.pi * V0))
    gs_cat = _f16((Gs8 @ Scat.T) * SCALE)            # [L, JC]

    # scalar path: mu_q, sig2 per (b, q, h)
    g2 = Gs8 @ np.stack([w_mu.astype(f8), w_sigma.astype(f8)], 1)   # [L,2]
    stg_all = np.empty((B, 2, K14, 2 * H * 512), np.float16)
    for b in range(B):
        t = k8[b].T @ g2                                            # [E,2]
        Wh = np.empty((E, H, 2), f8)
        for h in range(H):
            u_ = Wk.astype(f8)[h * D:(h + 1) * D, :] @ t * sD
            Wh[:, h, :] = Wq.astype(f8)[h * D:(h + 1) * D, :].T @ u_
        sv = np.einsum('qe,ehc->qhc', q8[b], Wh)                    # [Q,H,2]
        mu = 1.0 / (1.0 + np.exp(-sv[..., 0]))                      # [Q,H]
        sig2 = np.clip(np.logaddexp(0.0, sv[..., 1]), CLAMP_MIN, None)
        rows = np.empty((K14, Q, H), f8)
        for s in range(2):
            var = sig2 + SIGMAS[s] ** 2 - V0
            Ac = -0.5 * mu * mu / var - 0.5 * np.log(2 * np.pi * var)
            Bc = mu / var
            Cc = -0.5 / var
            Ah = np.float16(Ac).astype(f8); Al = Ac - Ah
            Bh = np.float16(Bc).astype(f8); Bl = Bc - Bh
            Ch = np.float16(Cc).astype(f8); Cl = Cc - Ch
            rows[s * 7:(s + 1) * 7] = np.stack([Ah, Al, Bh, Bl, Ch, Cl, Ch])
        r16 = rows.astype(np.float16)                               # [14, Q, H]
        # stg col = (qb*H + h)*512 + i ; q index within core = qb*512 + i
        for qh in range(2):
            for qb in range(QC // 512):
                sl = r16[:, qh * QC + qb * 512: qh * QC + (qb + 1) * 512, :]
                stg_all[b, qh, :, qb * H * 512:(qb + 1) * H * 512] = (
                    sl.transpose(0, 2, 1).reshape(K14, H * 512))
    return gs_cat, mks, stg_all


def kernel(k, q, Wq, Wk, Wv, Wo, w_mu, w_sigma, Gs, basis_mu, basis_sigma):
    k = np.ascontiguousarray(np.asarray(k, np.float32))
    q = np.ascontiguousarray(np.asarray(q, np.float32))
    gs_cat, mks, stg_all = _host_prep(
        k, q, np.asarray(Wq), np.asarray(Wk),
        np.asarray(w_mu), np.asarray(w_sigma),
        np.asarray(Gs), np.asarray(basis_mu))
    wvt = _f16(np.asarray(Wv, np.float32).T)
    wot = _f16(np.asarray(Wo, np.float32).T)

    nc = _build_nc()
    in_maps = []
    for c in range(8):
        b, qh = c // 2, c % 2
        in_maps.append({
            "kb": _f16(k[b]), "gsc": gs_cat, "wvt": wvt, "wot": wot,
            "stg": np.ascontiguousarray(stg_all[b, qh]),
            "mkp": mks, "idm": np.eye(P, dtype=np.float16),
        })
    trace = bool(os.environ.get("KERNEL_TRACE"))
    if trace:
        _install_ntff_shim()
    res = run_bass_kernel_spmd(nc, in_maps, list(range(8)), trace=trace)
    global LAST_EXEC_NS
    LAST_EXEC_NS = res.exec_time_ns
    out = np.empty((B, Q, E), np.float32)
    for c in range(8):
        b, qh = c // 2, c % 2
        out[b, qh * QC:(qh + 1) * QC, :] = res.results[c]["out"].astype(np.float32)
    out *= np.float32(1.0 / SCALE)
    return out
